# revision 1
# baseline (speedup 1.0000x reference)
"""CrossAttention TRN2 kernel: 8-core SPMD, shard = (batch, S1-half).

Per core: q rows [1024, 512] of one batch; full k,v [2048,512] of that batch;
all weights. Flash-style attention with S^T recompute (no P transpose):
  1. PE-transpose q,k,v -> qT,kT,vT (d on partitions), round to fp32r.
  2. Projections (fp32r): qhT_aug[65,1024]/khT_aug[65,2048] per head,
     vh[t,512] bf16 (all heads).
  3. Per head: raw-S max pass ([s,t] psum, DVE negated rowmax) -> aug row
     of qhT (-max, via HWDGE sbuf-to-sbuf DMA); S^T pass with K=65 (ones row in khT
     adds -max[s]); ACT exp(scale=1/8) -> P^T bf16; PV col-packed with
     ones-lhsT row-sum matmul (concurrent col group) accumulating
     oT[65, s] psum over 16 t-tiles.
  4. Per head: transpose oT+r -> [s, 64|r], reciprocal+scale (normalize),
     assemble out_norm [s, 512]; transpose back -> outT [hp, s];
     final proj vs Wo (bf16) -> out [1024, 512].
"""
import sys
import functools

sys.path.insert(0, "/opt/trn_rl_repo")
import numpy as np
from contextlib import ExitStack

B, S1, S2, D, H, P = 4, 2048, 2048, 512, 8, 64
SC = S1 // 2          # 1024 q rows per core
NCORES = 8
DCH = D // 128        # 4 d-chunks
QT = SC // 128        # 8 q s-tiles
TT = S2 // 128        # 16 t-tiles
TBLK = S2 // 512      # 4 t blocks of 512
SBL = SC // 512       # 2 s blocks of 512


@functools.lru_cache(maxsize=1)
def _build():
    from concourse import bacc, tile, mybir, masks

    f32 = mybir.dt.float32
    f32r = mybir.dt.float32r
    bf16 = mybir.dt.bfloat16

    nc = bacc.Bacc("TRN2", target_bir_lowering=False, debug=False)

    q_d = nc.dram_tensor("q", [SC, D], f32, kind="ExternalInput").ap()
    k_d = nc.dram_tensor("k", [S2, D], f32, kind="ExternalInput").ap()
    v_d = nc.dram_tensor("v", [S2, D], f32, kind="ExternalInput").ap()
    wq_d = nc.dram_tensor("Wq", [H, D, P], f32, kind="ExternalInput").ap()
    wk_d = nc.dram_tensor("Wk", [H, D, P], f32, kind="ExternalInput").ap()
    wv_d = nc.dram_tensor("Wv", [H, D, P], f32, kind="ExternalInput").ap()
    wo_d = nc.dram_tensor("Wo", [H * P, D], f32, kind="ExternalInput").ap()
    out_d = nc.dram_tensor("out", [SC, D], f32, kind="ExternalOutput").ap()

    with tile.TileContext(nc) as tc, ExitStack() as ctx:
        const_pool = ctx.enter_context(tc.tile_pool(name="const", bufs=1))
        ident = const_pool.tile([128, 128], f32)
        masks.make_identity(nc, ident[:])
        ident_bf = const_pool.tile([128, 128], bf16)
        masks.make_identity(nc, ident_bf[:])
        ones_bf = const_pool.tile([128, 1], bf16)
        nc.vector.memset(ones_bf[:], 1.0)

        # ---- weights: load fp32, round to fp32r on gpsimd / cast bf16 ----
        wpool = ctx.enter_context(tc.tile_pool(name="wr", bufs=1))
        wq_r = [wpool.tile([128, H * P], f32r, tag=f"wq{c}", name=f"wq{c}") for c in range(DCH)]
        wk_r = [wpool.tile([128, H * P], f32r, tag=f"wk{c}", name=f"wk{c}") for c in range(DCH)]
        wv_r = [wpool.tile([128, H * P], f32r, tag=f"wv{c}", name=f"wv{c}") for c in range(DCH)]
        wo_bf = [wpool.tile([128, D], bf16, tag=f"wo{c}", name=f"wo{c}") for c in range(DCH)]

        nat_pool = ctx.enter_context(tc.tile_pool(name="nat", bufs=8))
        act_pool = ctx.enter_context(tc.tile_pool(name="acts", bufs=1))
        qhT = [act_pool.tile([65, SC], f32r, tag=f"qhT{h}", name=f"qhT{h}") for h in range(H)]
        khT = [act_pool.tile([65, S2], f32r, tag=f"khT{h}", name=f"khT{h}") for h in range(H)]
        vh = [act_pool.tile([128, H * P], bf16, tag=f"vh{t}", name=f"vh{t}") for t in range(TT)]
        ones_row = const_pool.tile([1, S2], f32)
        nc.vector.memset(ones_row[:], 1.0)
        for h in range(H):
            nc.scalar.copy(khT[h][64:65, :], ones_row[:])

        def transpose_round(src_d, nrows, dstT):
            """src_d [nrows, D] fp32 DRAM -> dstT[c] [128, nrows] fp32r (c = d-chunk)."""
            with tc.tile_pool(name="tp_ps", bufs=2, space="PSUM") as tp_ps:
                ntile = nrows // 128
                for g in range(ntile // 4):
                    nats = []
                    for j in range(4):
                        si = 4 * g + j
                        nat = nat_pool.tile([128, D], f32, tag="nat")
                        nc.sync.dma_start(nat[:], src_d[si * 128:(si + 1) * 128, :])
                        nats.append(nat)
                    for c in range(DCH):
                        ps = tp_ps.tile([128, 512], f32)
                        for j in range(4):
                            nc.tensor.transpose(
                                ps[:, j * 128:(j + 1) * 128],
                                nats[j][:, c * 128:(c + 1) * 128],
                                ident[:],
                            )
                        nc.vector.tensor_copy(dstT[c][:, g * 512:(g + 1) * 512], ps[:])

        # ---- q path ----
        with tc.tile_pool(name="qT", bufs=1) as qT_pool, \
             tc.tile_pool(name="proj_ps", bufs=3, space="PSUM") as proj_ps:
            qT = [qT_pool.tile([128, SC], f32r, tag=f"qT{c}", name=f"qT{c}") for c in range(DCH)]
            transpose_round(q_d, SC, qT)
            with tc.tile_pool(name="wtmp", bufs=1) as wtmp_pool:
                for name_d, dst in ((wq_d, wq_r), (wk_d, wk_r), (wv_d, wv_r)):
                    for c in range(DCH):
                        wt = wtmp_pool.tile([128, H * P], f32, tag=f"wt{c}")
                        for h in range(H):
                            nc.sync.dma_start(
                                wt[:, h * P:(h + 1) * P],
                                name_d[h, c * 128:(c + 1) * 128, :],
                            )
                        nc.gpsimd.tensor_copy(dst[c][:], wt[:])
                for c in range(DCH):
                    wt = wtmp_pool.tile([128, D], f32, tag=f"wtmp_o{c}")
                    nc.sync.dma_start(wt[:], wo_d[c * 128:(c + 1) * 128, :])
                    nc.gpsimd.tensor_copy(wo_bf[c][:], wt[:])
            for hp in range(H // 2):
                for sb in range(SBL):
                    ps = proj_ps.tile([128, 512], f32)
                    for c in range(DCH):
                        nc.tensor.matmul(
                            ps[:],
                            wq_r[c][:, hp * 128:(hp + 1) * 128],
                            qT[c][:, sb * 512:(sb + 1) * 512],
                            start=(c == 0), stop=(c == DCH - 1),
                        )
                    eng = nc.scalar.copy if sb == 0 else nc.vector.tensor_copy
                    eng(qhT[2 * hp][0:64, sb * 512:(sb + 1) * 512], ps[0:64, :])
                    eng(qhT[2 * hp + 1][0:64, sb * 512:(sb + 1) * 512], ps[64:128, :])

        # ---- k path ----
        with tc.tile_pool(name="kT", bufs=1) as kT_pool, \
             tc.tile_pool(name="proj_ps2", bufs=3, space="PSUM") as proj_ps:
            kT = [kT_pool.tile([128, S2], f32r, tag=f"kT{c}", name=f"kT{c}") for c in range(DCH)]
            transpose_round(k_d, S2, kT)
            for hp in range(H // 2):
                for tb in range(TBLK):
                    ps = proj_ps.tile([128, 512], f32)
                    for c in range(DCH):
                        nc.tensor.matmul(
                            ps[:],
                            wk_r[c][:, hp * 128:(hp + 1) * 128],
                            kT[c][:, tb * 512:(tb + 1) * 512],
                            start=(c == 0), stop=(c == DCH - 1),
                        )
                    eng = nc.scalar.copy if tb % 2 == 0 else nc.vector.tensor_copy
                    eng(khT[2 * hp][0:64, tb * 512:(tb + 1) * 512], ps[0:64, :])
                    eng(khT[2 * hp + 1][0:64, tb * 512:(tb + 1) * 512], ps[64:128, :])

        # ---- v path ----
        with tc.tile_pool(name="vT", bufs=1) as vT_pool, \
             tc.tile_pool(name="proj_ps3", bufs=3, space="PSUM") as proj_ps:
            vT = [vT_pool.tile([128, S2], f32r, tag=f"vT{c}", name=f"vT{c}") for c in range(DCH)]
            transpose_round(v_d, S2, vT)
            for ti in range(TT):
                ps = proj_ps.tile([128, 512], f32)
                for c in range(DCH):
                    nc.tensor.matmul(
                        ps[:],
                        vT[c][:, ti * 128:(ti + 1) * 128],
                        wv_r[c][:],
                        start=(c == 0), stop=(c == DCH - 1),
                    )
                nc.vector.tensor_copy(vh[ti][:], ps[:])

        # ---- attention per head ----
        fin_pool = ctx.enter_context(tc.tile_pool(name="fin", bufs=1))
        out_norm = [fin_pool.tile([128, H * P], bf16, tag=f"onorm{sc}", name=f"onorm{sc}") for sc in range(QT)]
        outT = [fin_pool.tile([128, SC], bf16, tag=f"outT{c}", name=f"outT{c}") for c in range(DCH)]

        with tc.tile_pool(name="max_ps", bufs=1, space="PSUM") as max_ps, \
             tc.tile_pool(name="st_ps", bufs=2, space="PSUM") as st_ps, \
             tc.tile_pool(name="oT_ps", bufs=2, space="PSUM") as oT_ps, \
             tc.tile_pool(name="pt", bufs=3) as pt_pool, \
             tc.tile_pool(name="small", bufs=8) as small_pool, \
             tc.tile_pool(name="oT_sb", bufs=2) as oT_sb_pool:
            def maxpass_qi(h, qi):
                negm = small_pool.tile([128, 1], f32, tag="negm", name=f"negm{h}_{qi}")
                tmp = small_pool.tile([128, 1], f32, tag="tmpm", name=f"tmpm{h}_{qi}")
                negm_r = small_pool.tile([128, 1], f32r, tag="negmr", name=f"negmr{h}_{qi}")
                for half in range(2):
                    ps = max_ps.tile([128, 1024], f32, tag="mx", name=f"mx{h}_{qi}_{half}")
                    for tb in range(2):
                        nc.tensor.matmul(
                            ps[:, tb * 512:(tb + 1) * 512],
                            qhT[h][0:64, qi * 128:(qi + 1) * 128],
                            khT[h][0:64, (2 * half + tb) * 512:(2 * half + tb + 1) * 512],
                            start=True, stop=True,
                        )
                    dst = negm if half == 0 else tmp
                    nc.vector.tensor_reduce(
                        dst[:], ps[:], axis=mybir.AxisListType.X,
                        op=mybir.AluOpType.max, negate=True,
                    )
                nc.vector.tensor_scalar_min(negm[:], tmp[:], negm[:])
                nc.vector.tensor_copy(negm_r[:], negm[:])
                nc.sync.dma_start(
                    qhT[h][64:65, qi * 128:(qi + 1) * 128], negm_r[:],
                )

            for qi in range(QT):
                maxpass_qi(0, qi)

            for h in range(H):
                # --- S^T + exp + PV/rowsum, next head's max pass interleaved ---
                oTs = [oT_ps.tile([65, 512], f32, tag="oT", name=f"oT{h}_{_sb}") for _sb in range(SBL)]
                for ti in range(TT):
                    st = st_ps.tile([128, 1024], f32, tag="st", name=f"st{h}_{ti}")
                    for sb in range(SBL):
                        nc.tensor.matmul(
                            st[:, sb * 512:(sb + 1) * 512],
                            khT[h][0:65, ti * 128:(ti + 1) * 128],
                            qhT[h][0:65, sb * 512:(sb + 1) * 512],
                            start=True, stop=True,
                        )
                    ptile = pt_pool.tile([128, 1024], bf16, tag="pt", name=f"pt{h}_{ti}")
                    nc.scalar.activation(ptile[:], st[:], mybir.ActivationFunctionType.Exp,
                                         scale=0.125)
                    for sb in range(SBL):
                        nc.tensor.matmul(
                            oTs[sb][0:64, :],
                            vh[ti][:, h * P:(h + 1) * P],
                            ptile[:, sb * 512:(sb + 1) * 512],
                            start=(ti == 0), stop=(ti == TT - 1),
                            tile_position=(0, 0),
                        )
                        nc.tensor.matmul(
                            oTs[sb][64:65, :],
                            ones_bf[:],
                            ptile[:, sb * 512:(sb + 1) * 512],
                            start=(ti == 0), stop=(ti == TT - 1),
                            tile_position=(0, 64),
                        )
                    if h + 1 < H and ti < QT:
                        maxpass_qi(h + 1, ti)

                # --- stage oT, transpose, normalize into out_norm ---
                oT_h = oT_sb_pool.tile([65, SC], f32, tag="oT_h", name=f"oTh{h}")
                for sb in range(SBL):
                    nc.vector.tensor_copy(oT_h[:, sb * 512:(sb + 1) * 512], oTs[sb][:])
                for sc in range(QT):
                    tps = oT_ps.tile([128, 128], f32, tag="oT", name=f"tps{h}_{sc}")
                    nc.tensor.transpose(
                        tps[0:128, 0:65],
                        oT_h[:, sc * 128:(sc + 1) * 128],
                        ident[0:65, 0:65],
                    )
                    rec = small_pool.tile([128, 1], f32, tag="rec", name=f"rec{h}_{sc}")
                    nc.vector.reciprocal(rec[:], tps[:, 64:65])
                    nc.vector.tensor_scalar_mul(
                        out_norm[sc][:, h * P:(h + 1) * P], tps[:, 0:64], rec[:],
                    )
                if h % 2 == 1:
                    c = h // 2
                    for sc in range(QT):
                        tps2 = oT_ps.tile([128, 128], bf16, tag="oT", name=f"tb{c}_{sc}")
                        nc.tensor.transpose(
                            tps2[:], out_norm[sc][:, c * 128:(c + 1) * 128], ident_bf[:],
                        )
                        nc.scalar.copy(outT[c][:, sc * 128:(sc + 1) * 128], tps2[:])

        # ---- final projection ----
        with tc.tile_pool(name="fin_ps", bufs=2, space="PSUM") as fin_ps, \
             tc.tile_pool(name="fin_sb", bufs=2) as fin_sb_pool:
            for sc in range(QT):
                ps = fin_ps.tile([128, 512], f32, tag="fps", name=f"fps{sc}")
                for c in range(DCH):
                    nc.tensor.matmul(
                        ps[:],
                        outT[c][:, sc * 128:(sc + 1) * 128],
                        wo_bf[c][:],
                        start=(c == 0), stop=(c == DCH - 1),
                    )
                fin = fin_sb_pool.tile([128, 512], f32, tag="fin", name=f"fin{sc}")
                nc.vector.tensor_copy(fin[:], ps[:])
                nc.sync.dma_start(out_d[sc * 128:(sc + 1) * 128, :], fin[:])

    nc.compile()
    return nc


def kernel(q, k, v, Wq, Wk, Wv, Wo):
    nc = _build()
    from concourse.bass_utils import run_bass_kernel_spmd

    q = np.asarray(q, np.float32)
    k = np.asarray(k, np.float32)
    v = np.asarray(v, np.float32)
    in_maps = []
    for c in range(NCORES):
        b, half = c // 2, c % 2
        in_maps.append({
            "q": np.ascontiguousarray(q[b, half * SC:(half + 1) * SC, :]),
            "k": np.ascontiguousarray(k[b]),
            "v": np.ascontiguousarray(v[b]),
            "Wq": np.ascontiguousarray(Wq, dtype=np.float32),
            "Wk": np.ascontiguousarray(Wk, dtype=np.float32),
            "Wv": np.ascontiguousarray(Wv, dtype=np.float32),
            "Wo": np.ascontiguousarray(Wo, dtype=np.float32),
        })
    res = run_bass_kernel_spmd(nc, in_maps, core_ids=list(range(NCORES)))
    globals()["LAST_RES"] = res
    out = np.empty((B, S1, D), np.float32)
    for c, r in enumerate(res.results):
        b, half = c // 2, c % 2
        out[b, half * SC:(half + 1) * SC] = r["out"]
    return out


if __name__ == "__main__":
    rng = np.random.default_rng(0)
    qq = rng.standard_normal((B, S1, D), dtype=np.float32)
    kk = rng.standard_normal((B, S2, D), dtype=np.float32)
    vv = rng.standard_normal((B, S2, D), dtype=np.float32)
    wq = rng.standard_normal((H, D, P), dtype=np.float32)
    wk = rng.standard_normal((H, D, P), dtype=np.float32)
    wv = rng.standard_normal((H, D, P), dtype=np.float32)
    wo = rng.standard_normal((H * P, D), dtype=np.float32)
    o = kernel(qq, kk, vv, wq, wk, wv, wo)
    print("out", o.shape, o.dtype, np.abs(o).mean())



# revision 26
# speedup vs baseline: 1.2506x; 1.2506x over previous
"""CrossAttention TRN2 kernel: 8-core SPMD, shard = (batch, S1-half).

Per core: q rows [1024, 512] of one batch; full k,v [2048,512] of that batch;
all weights. Flash-style attention with S^T recompute (no P transpose):
  1. PE-transpose q,k,v -> qT,kT,vT (d on partitions), round to fp32r.
  2. Projections (fp32r): qhT_aug[65,1024]/khT_aug[65,2048] per head,
     vh_aug[t, 8*65] bf16 (per-head 64 cols + ones col for rowsum).
  3. Per head: raw-S max pass ([s,t] psum in 4 512-col chunks, rowmax
     reduces split Pool/DVE, negated, min-combined) -> aug row of qhT
     (-max, via HWDGE sbuf-to-sbuf DMA); S^T pass with K=65 (ones row in
     khT adds -max[s]); ACT exp(scale=1/8) -> P^T bf16; PV with augmented
     vh lhsT [128,65] accumulating oT[65, s] psum over 16 t-tiles (row 64
     = rowsum, no extra matmul).
  4. Per head: reciprocal of rowsum row in oT_h; transpose oT -> [s, 65]
     (col 64 = 1/rowsum); tensor_scalar_mul normalize into out_norm
     [s, 512]; transpose back -> outT [hp, s]; final proj vs Wo (bf16).
Engine budget: PE ~233us floor; exp on ACT only; copies/reduces split
Pool/DVE to keep both under PE.
"""
import sys
import functools

sys.path.insert(0, "/opt/trn_rl_repo")
import numpy as np
from contextlib import ExitStack

B, S1, S2, D, H, P = 4, 2048, 2048, 512, 8, 64
SC = S1 // 2          # 1024 q rows per core
NCORES = 8
DCH = D // 128        # 4 d-chunks
QT = SC // 128        # 8 q s-tiles
TT = S2 // 128        # 16 t-tiles
TBLK = S2 // 512      # 4 t blocks of 512
SBL = SC // 512       # 2 s blocks of 512
PA = P + 1            # 65: per-head vh block width (64 vals + ones col)


@functools.lru_cache(maxsize=1)
def _build():
    from concourse import bacc, tile, mybir, masks

    f32 = mybir.dt.float32
    f32r = mybir.dt.float32r
    bf16 = mybir.dt.bfloat16

    nc = bacc.Bacc("TRN2", target_bir_lowering=False, debug=False)

    q_d = nc.dram_tensor("q", [SC, D], f32, kind="ExternalInput").ap()
    k_d = nc.dram_tensor("k", [S2, D], f32, kind="ExternalInput").ap()
    v_d = nc.dram_tensor("v", [S2, D], f32, kind="ExternalInput").ap()
    wq_d = nc.dram_tensor("Wq", [H, D, P], f32, kind="ExternalInput").ap()
    wk_d = nc.dram_tensor("Wk", [H, D, P], f32, kind="ExternalInput").ap()
    wv_d = nc.dram_tensor("Wv", [H, D, P], f32, kind="ExternalInput").ap()
    wo_d = nc.dram_tensor("Wo", [H * P, D], f32, kind="ExternalInput").ap()
    out_d = nc.dram_tensor("out", [SC, D], f32, kind="ExternalOutput").ap()

    with tile.TileContext(nc) as tc, ExitStack() as ctx:
        const_pool = ctx.enter_context(tc.tile_pool(name="const", bufs=1))
        ident = const_pool.tile([128, 128], f32)
        masks.make_identity(nc, ident[:])
        ident_bf = const_pool.tile([128, 128], bf16)
        masks.make_identity(nc, ident_bf[:])

        # ---- weights: load fp32, round to fp32r / cast bf16 on gpsimd ----
        # wv_r/wo_bf persist (used in v path / final proj); wq_r/wk_r live in
        # a scoped pool freed before the v path to fit vT in SBUF.
        wpool = ctx.enter_context(tc.tile_pool(name="wr", bufs=1))
        wv_r = [wpool.tile([128, H * P], f32r, tag=f"wv{c}", name=f"wv{c}") for c in range(DCH)]
        wo_bf = [wpool.tile([128, D], bf16, tag=f"wo{c}", name=f"wo{c}") for c in range(DCH)]
        act_pool = ctx.enter_context(tc.tile_pool(name="acts", bufs=1))
        qhT = [act_pool.tile([65, SC], f32r, tag=f"qhT{h}", name=f"qhT{h}") for h in range(H)]
        khT = [act_pool.tile([65, S2], f32r, tag=f"khT{h}", name=f"khT{h}") for h in range(H)]
        vh = [act_pool.tile([128, H * PA], bf16, tag=f"vh{t}", name=f"vh{t}") for t in range(TT)]
        # ones columns of augmented vh (col 64 of each per-head 65-block)
        for t in range(TT):
            nc.gpsimd.memset(
                vh[t].rearrange("p (h x) -> p h x", h=H)[:, :, P:PA], 1.0,
            )
        # max-pass pools (persist into the attention phase) must be created
        # before the prologue-scoped pools for LIFO pool release
        max_ps = ctx.enter_context(tc.tile_pool(name="max_ps", bufs=1, space="PSUM"))
        small_pool = ctx.enter_context(tc.tile_pool(name="small", bufs=8))
        # prologue-scoped pools, closed explicitly to free SBUF for later phases
        ppool_es = ExitStack()
        nat_pool = ppool_es.enter_context(tc.tile_pool(name="nat", bufs=6))
        wqk_es = ExitStack()
        wqk_pool = wqk_es.enter_context(tc.tile_pool(name="wqk", bufs=1))
        wtmp_pool = wqk_es.enter_context(tc.tile_pool(name="wtmp", bufs=1))
        wq_r = [wqk_pool.tile([128, H * P], f32r, tag=f"wq{c}", name=f"wq{c}") for c in range(DCH)]
        wk_r = [wqk_pool.tile([128, H * P], f32r, tag=f"wk{c}", name=f"wk{c}") for c in range(DCH)]
        ones_row = wtmp_pool.tile([1, S2], bf16, tag="ones", name="ones_row")
        nc.vector.memset(ones_row[:], 1.0)
        for h in range(H):
            nc.scalar.copy(khT[h][64:65, :], ones_row[:])

        def transpose_round(src_d, nrows, dstT):
            """src_d [nrows, D] fp32 DRAM -> dstT[c] [128, nrows] fp32r (c = d-chunk)."""
            with tc.tile_pool(name="tp_ps", bufs=2, space="PSUM") as tp_ps:
                ntile = nrows // 128
                for g in range(ntile // 4):
                    nats = []
                    for j in range(4):
                        si = 4 * g + j
                        nat = nat_pool.tile([128, D], f32, tag="nat")
                        nc.sync.dma_start(nat[:], src_d[si * 128:(si + 1) * 128, :])
                        nats.append(nat)
                    for c in range(DCH):
                        ps = tp_ps.tile([128, 512], f32)
                        for j in range(4):
                            nc.tensor.transpose(
                                ps[:, j * 128:(j + 1) * 128],
                                nats[j][:, c * 128:(c + 1) * 128],
                                ident[:],
                            )
                        eng = nc.vector.tensor_copy if c % 2 == 0 else nc.scalar.copy
                        eng(dstT[c][:, g * 512:(g + 1) * 512], ps[:])

        def load_w(name_d, dst):
            for c in range(DCH):
                wt = wtmp_pool.tile([128, H * P], f32, tag=f"wt{c}", name=f"wt{c}")
                nc.sync.dma_start(
                    wt[:].rearrange("p (h x) -> p h x", h=H),
                    name_d[:, c * 128:(c + 1) * 128, :].transpose([1, 0, 2]),
                )
                nc.gpsimd.tensor_copy(dst[c][:], wt[:])

        # ---- q path ----
        with tc.tile_pool(name="qT", bufs=1) as qT_pool, \
             tc.tile_pool(name="proj_ps", bufs=3, space="PSUM") as proj_ps:
            qT = [qT_pool.tile([128, SC], f32r, tag=f"qT{c}", name=f"qT{c}") for c in range(DCH)]
            transpose_round(q_d, SC, qT)
            load_w(wq_d, wq_r)
            for hp in range(H // 2):
                for sb in range(SBL):
                    ps = proj_ps.tile([128, 512], f32)
                    for c in range(DCH):
                        nc.tensor.matmul(
                            ps[:],
                            wq_r[c][:, hp * 128:(hp + 1) * 128],
                            qT[c][:, sb * 512:(sb + 1) * 512],
                            start=(c == 0), stop=(c == DCH - 1),
                        )
                    eng = nc.scalar.copy if sb == 0 else nc.vector.tensor_copy
                    eng(qhT[2 * hp][0:64, sb * 512:(sb + 1) * 512], ps[0:64, :])
                    eng(qhT[2 * hp + 1][0:64, sb * 512:(sb + 1) * 512], ps[64:128, :])

        # ---- k path ----
        with tc.tile_pool(name="kT", bufs=1) as kT_pool, \
             tc.tile_pool(name="proj_ps2", bufs=3, space="PSUM") as proj_ps:
            kT = [kT_pool.tile([128, S2], f32r, tag=f"kT{c}", name=f"kT{c}") for c in range(DCH)]
            transpose_round(k_d, S2, kT)
            load_w(wk_d, wk_r)
            load_w(wv_d, wv_r)
            for c in range(DCH):
                wt = wtmp_pool.tile([128, D], f32, tag=f"wt{c}", name=f"wto{c}")
                nc.sync.dma_start(wt[:], wo_d[c * 128:(c + 1) * 128, :])
                nc.gpsimd.tensor_copy(wo_bf[c][:], wt[:])
            for hp in range(H // 2):
                for tb in range(TBLK):
                    ps = proj_ps.tile([128, 512], f32)
                    for c in range(DCH):
                        nc.tensor.matmul(
                            ps[:],
                            wk_r[c][:, hp * 128:(hp + 1) * 128],
                            kT[c][:, tb * 512:(tb + 1) * 512],
                            start=(c == 0), stop=(c == DCH - 1),
                        )
                    eng = nc.scalar.copy if tb % 2 == 0 else nc.vector.tensor_copy
                    eng(khT[2 * hp][0:64, tb * 512:(tb + 1) * 512], ps[0:64, :])
                    eng(khT[2 * hp + 1][0:64, tb * 512:(tb + 1) * 512], ps[64:128, :])
        wqk_es.close()

        # ---- hybrid max pass machinery ----
        # t-half-0 ([0,1024)): raw S [s,t] chunks, DVE rowmax (negated).
        # t-half-1 ([1024,2048)): raw S^T [t,s] tiles, Pool partition-max
        # (axis=C) into stack rows; one stacked Pool reduce -> -max strip.
        # One slot per ti: raw-S [s,t] psum [128,1024] half, DVE negated rowmax;
        # after the odd half, min-combine (fused f32r cast) and DMA the
        # [128,1]->[1,128] strip into qhT's aug row.
        def mp_begin(h):
            return {}

        def mp_step(h, mp, ti):
            if ti >= 2 * QT:
                return
            qi, half = ti // 2, ti % 2
            ps = max_ps.tile([128, 1024], f32, tag="mx", name=f"mx{h}_{qi}_{half}")
            for tb in range(2):
                nc.tensor.matmul(
                    ps[:, tb * 512:(tb + 1) * 512],
                    qhT[h][0:64, qi * 128:(qi + 1) * 128],
                    khT[h][0:64, (2 * half + tb) * 512:(2 * half + tb + 1) * 512],
                    start=True, stop=True,
                )
            dst = small_pool.tile([128, 1], f32, tag=f"ng{half}", name=f"ng{half}_{h}_{qi}")
            nc.vector.tensor_reduce(
                dst[:], ps[:], axis=mybir.AxisListType.X,
                op=mybir.AluOpType.max, negate=True,
            )
            mp[(qi, half)] = dst
            if half == 1:
                negm_r = small_pool.tile([128, 1], f32r, tag="negmr", name=f"negmr{h}_{qi}")
                nc.vector.tensor_scalar_min(negm_r[:], mp[(qi, 0)][:], dst[:])
                nc.sync.dma_start(
                    qhT[h][64:65, qi * 128:(qi + 1) * 128], negm_r[:],
                )

        # ---- v path (head 0 max pass interleaved) ----
        mp0 = mp_begin(0)
        with tc.tile_pool(name="vT", bufs=1) as vT_pool, \
             tc.tile_pool(name="proj_ps3", bufs=3, space="PSUM") as proj_ps:
            vT = [vT_pool.tile([128, S2], f32r, tag=f"vT{c}", name=f"vT{c}") for c in range(DCH)]
            transpose_round(v_d, S2, vT)
            for ti in range(TT):
                ps = proj_ps.tile([128, 512], f32)
                for c in range(DCH):
                    nc.tensor.matmul(
                        ps[:],
                        vT[c][:, ti * 128:(ti + 1) * 128],
                        wv_r[c][:],
                        start=(c == 0), stop=(c == DCH - 1),
                    )
                eng = nc.vector.tensor_copy if ti % 2 == 0 else nc.scalar.copy
                eng(
                    vh[ti].rearrange("p (h x) -> p h x", h=H)[:, :, 0:P],
                    ps[:].rearrange("p (h x) -> p h x", h=H),
                )
                mp_step(0, mp0, ti)
        ppool_es.close()

        # ---- attention per head ----
        fin_pool = ctx.enter_context(tc.tile_pool(name="fin", bufs=1))
        out_norm = [fin_pool.tile([128, H * P], bf16, tag=f"onorm{sc}", name=f"onorm{sc}") for sc in range(QT)]
        outT = [fin_pool.tile([128, SC], bf16, tag=f"outT{c}", name=f"outT{c}") for c in range(DCH)]

        with tc.tile_pool(name="st_ps", bufs=2, space="PSUM") as st_ps, \
             tc.tile_pool(name="oT_ps", bufs=2, space="PSUM") as oT_ps, \
             tc.tile_pool(name="pt", bufs=3) as pt_pool, \
             tc.tile_pool(name="oT_sb", bufs=2) as oT_sb_pool:
            for h in range(H):
                # --- S^T + exp + PV, next head's max pass interleaved ---
                if h + 1 < H:
                    mp_next = mp_begin(h + 1)
                oTs = [oT_ps.tile([65, 512], f32, tag="oT", name=f"oT{h}_{_sb}") for _sb in range(SBL)]
                for ti in range(TT):
                    st = st_ps.tile([128, 1024], f32, tag="st", name=f"st{h}_{ti}")
                    for sb in range(SBL):
                        nc.tensor.matmul(
                            st[:, sb * 512:(sb + 1) * 512],
                            khT[h][0:65, ti * 128:(ti + 1) * 128],
                            qhT[h][0:65, sb * 512:(sb + 1) * 512],
                            start=True, stop=True,
                        )
                    ptile = pt_pool.tile([128, 1024], bf16, tag="pt", name=f"pt{h}_{ti}")
                    nc.scalar.activation(ptile[:], st[:], mybir.ActivationFunctionType.Exp,
                                         scale=0.125)
                    for sb in range(SBL):
                        nc.tensor.matmul(
                            oTs[sb][0:65, :],
                            vh[ti][:, h * PA:(h + 1) * PA],
                            ptile[:, sb * 512:(sb + 1) * 512],
                            start=(ti == 0), stop=(ti == TT - 1),
                        )
                    if h + 1 < H:
                        mp_step(h + 1, mp_next, ti)

                # --- stage oT, reciprocal of rowsum row, transpose, normalize ---
                # pipelined per 512-col half so PE transposes of half 0 overlap
                # the copy+reciprocal of half 1; reciprocal reads the psum
                # rowsum row directly so it runs concurrently with the copy
                oT_h = oT_sb_pool.tile([65, SC], f32, tag="oT_h", name=f"oTh{h}")
                for sb in range(SBL):
                    sl = slice(sb * 512, (sb + 1) * 512)
                    nc.scalar.copy(oT_h[0:64, sl], oTs[sb][0:64, :])
                    nc.vector.reciprocal(oT_h[64:65, sl], oTs[sb][64:65, :])
                    for sc in range(sb * (QT // 2), (sb + 1) * (QT // 2)):
                        tps = oT_ps.tile([128, 128], f32, tag="oT", name=f"tps{h}_{sc}")
                        nc.tensor.transpose(
                            tps[0:128, 0:65],
                            oT_h[:, sc * 128:(sc + 1) * 128],
                            ident[0:65, 0:65],
                        )
                        nc.vector.tensor_scalar_mul(
                            out_norm[sc][:, h * P:(h + 1) * P], tps[:, 0:64], tps[:, 64:65],
                        )
                        if h % 2 == 1:
                            c = h // 2
                            tps2 = oT_ps.tile([128, 128], bf16, tag="oT", name=f"tb{c}_{sc}")
                            nc.tensor.transpose(
                                tps2[:], out_norm[sc][:, c * 128:(c + 1) * 128], ident_bf[:],
                            )
                            eng = nc.vector.tensor_copy if sc % 2 == 0 else nc.scalar.copy
                            eng(outT[c][:, sc * 128:(sc + 1) * 128], tps2[:])

        # ---- final projection ----
        with tc.tile_pool(name="fin_ps", bufs=2, space="PSUM") as fin_ps, \
             tc.tile_pool(name="fin_sb", bufs=2) as fin_sb_pool:
            for sc in range(QT):
                ps = fin_ps.tile([128, 512], f32, tag="fps", name=f"fps{sc}")
                for c in range(DCH):
                    nc.tensor.matmul(
                        ps[:],
                        outT[c][:, sc * 128:(sc + 1) * 128],
                        wo_bf[c][:],
                        start=(c == 0), stop=(c == DCH - 1),
                    )
                fin = fin_sb_pool.tile([128, 512], f32, tag="fin", name=f"fin{sc}")
                eng = nc.vector.tensor_copy if sc % 2 == 0 else nc.scalar.copy
                eng(fin[:], ps[:])
                nc.sync.dma_start(out_d[sc * 128:(sc + 1) * 128, :], fin[:])

    nc.compile()
    return nc


def kernel(q, k, v, Wq, Wk, Wv, Wo):
    nc = _build()
    from concourse.bass_utils import run_bass_kernel_spmd

    q = np.asarray(q, np.float32)
    k = np.asarray(k, np.float32)
    v = np.asarray(v, np.float32)
    in_maps = []
    for c in range(NCORES):
        b, half = c // 2, c % 2
        in_maps.append({
            "q": np.ascontiguousarray(q[b, half * SC:(half + 1) * SC, :]),
            "k": np.ascontiguousarray(k[b]),
            "v": np.ascontiguousarray(v[b]),
            "Wq": np.ascontiguousarray(Wq, dtype=np.float32),
            "Wk": np.ascontiguousarray(Wk, dtype=np.float32),
            "Wv": np.ascontiguousarray(Wv, dtype=np.float32),
            "Wo": np.ascontiguousarray(Wo, dtype=np.float32),
        })
    res = run_bass_kernel_spmd(nc, in_maps, core_ids=list(range(NCORES)))
    globals()["LAST_RES"] = res
    out = np.empty((B, S1, D), np.float32)
    for c, r in enumerate(res.results):
        b, half = c // 2, c % 2
        out[b, half * SC:(half + 1) * SC] = r["out"]
    return out


if __name__ == "__main__":
    rng = np.random.default_rng(0)
    qq = rng.standard_normal((B, S1, D), dtype=np.float32)
    kk = rng.standard_normal((B, S2, D), dtype=np.float32)
    vv = rng.standard_normal((B, S2, D), dtype=np.float32)
    wq = rng.standard_normal((H, D, P), dtype=np.float32)
    wk = rng.standard_normal((H, D, P), dtype=np.float32)
    wv = rng.standard_normal((H, D, P), dtype=np.float32)
    wo = rng.standard_normal((H * P, D), dtype=np.float32)
    o = kernel(qq, kk, vv, wq, wk, wv, wo)
    print("out", o.shape, o.dtype, np.abs(o).mean())


# revision 27
# speedup vs baseline: 1.3089x; 1.0466x over previous
"""CrossAttention TRN2 kernel: 8-core SPMD, shard = (batch, S1-half).

Per core: q rows [1024, 512] of one batch; full k,v [2048,512] of that batch;
all weights. Flash-style attention with S^T recompute (no P transpose):
  1. PE-transpose q,k,v -> qT,kT,vT (d on partitions), round to fp32r.
  2. Projections (fp32r): qhT_aug[65,1024]/khT_aug[65,2048] per head,
     vh_aug[t, 8*65] bf16 (per-head 64 cols + ones col for rowsum).
  3. Per head: raw-S max pass ([s,t] psum in 4 512-col chunks, rowmax
     reduces split Pool/DVE, negated, min-combined) -> aug row of qhT
     (-max, via HWDGE sbuf-to-sbuf DMA); S^T pass with K=65 (ones row in
     khT adds -max[s]); ACT exp(scale=1/8) -> P^T bf16; PV with augmented
     vh lhsT [128,65] accumulating oT[65, s] psum over 16 t-tiles (row 64
     = rowsum, no extra matmul).
  4. Per head: reciprocal of rowsum row in oT_h; transpose oT -> [s, 65]
     (col 64 = 1/rowsum); tensor_scalar_mul normalize into out_norm
     [s, 512]; transpose back -> outT [hp, s]; final proj vs Wo (bf16).
Engine budget: PE ~233us floor; exp on ACT only; copies/reduces split
Pool/DVE to keep both under PE.
"""
import sys
import functools

sys.path.insert(0, "/opt/trn_rl_repo")
import numpy as np
from contextlib import ExitStack

B, S1, S2, D, H, P = 4, 2048, 2048, 512, 8, 64
SC = S1 // 2          # 1024 q rows per core
NCORES = 8
DCH = D // 128        # 4 d-chunks
QT = SC // 128        # 8 q s-tiles
TT = S2 // 128        # 16 t-tiles
TBLK = S2 // 512      # 4 t blocks of 512
SBL = SC // 512       # 2 s blocks of 512
PA = P + 1            # 65: per-head vh block width (64 vals + ones col)


@functools.lru_cache(maxsize=1)
def _build():
    from concourse import bacc, tile, mybir, masks

    f32 = mybir.dt.float32
    f32r = mybir.dt.float32r
    bf16 = mybir.dt.bfloat16

    nc = bacc.Bacc("TRN2", target_bir_lowering=False, debug=False)

    q_d = nc.dram_tensor("q", [SC, D], f32, kind="ExternalInput").ap()
    k_d = nc.dram_tensor("k", [S2, D], f32, kind="ExternalInput").ap()
    v_d = nc.dram_tensor("v", [S2, D], f32, kind="ExternalInput").ap()
    wq_d = nc.dram_tensor("Wq", [H, D, P], f32, kind="ExternalInput").ap()
    wk_d = nc.dram_tensor("Wk", [H, D, P], f32, kind="ExternalInput").ap()
    wv_d = nc.dram_tensor("Wv", [H, D, P], f32, kind="ExternalInput").ap()
    wo_d = nc.dram_tensor("Wo", [H * P, D], f32, kind="ExternalInput").ap()
    out_d = nc.dram_tensor("out", [SC, D], f32, kind="ExternalOutput").ap()

    with tile.TileContext(nc) as tc, ExitStack() as ctx:
        const_pool = ctx.enter_context(tc.tile_pool(name="const", bufs=1))
        ident = const_pool.tile([128, 128], f32)
        masks.make_identity(nc, ident[:])
        ident_bf = const_pool.tile([128, 128], bf16)
        masks.make_identity(nc, ident_bf[:])

        # ---- weights: load fp32, round to fp32r / cast bf16 on gpsimd ----
        # wv_r/wo_bf persist (used in v path / final proj); wq_r/wk_r live in
        # a scoped pool freed before the v path to fit vT in SBUF.
        wpool = ctx.enter_context(tc.tile_pool(name="wr", bufs=1))
        wv_r = [wpool.tile([128, H * P], f32r, tag=f"wv{c}", name=f"wv{c}") for c in range(DCH)]
        wo_bf = [wpool.tile([128, D], bf16, tag=f"wo{c}", name=f"wo{c}") for c in range(DCH)]
        act_pool = ctx.enter_context(tc.tile_pool(name="acts", bufs=1))
        qhT = [act_pool.tile([65, SC], f32r, tag=f"qhT{h}", name=f"qhT{h}") for h in range(H)]
        khT = [act_pool.tile([65, S2], f32r, tag=f"khT{h}", name=f"khT{h}") for h in range(H)]
        vh = [act_pool.tile([128, H * PA], bf16, tag=f"vh{t}", name=f"vh{t}") for t in range(TT)]
        # ones columns of augmented vh (col 64 of each per-head 65-block)
        for t in range(TT):
            nc.gpsimd.memset(
                vh[t].rearrange("p (h x) -> p h x", h=H)[:, :, P:PA], 1.0,
            )
        # max-pass pools (persist into the attention phase) must be created
        # before the prologue-scoped pools for LIFO pool release
        small_pool = ctx.enter_context(tc.tile_pool(name="small", bufs=8))
        # prologue-scoped pools, closed explicitly to free SBUF for later phases
        ppool_es = ExitStack()
        nat_pool = ppool_es.enter_context(tc.tile_pool(name="nat", bufs=6))
        wqk_es = ExitStack()
        wqk_pool = wqk_es.enter_context(tc.tile_pool(name="wqk", bufs=1))
        wtmp_pool = wqk_es.enter_context(tc.tile_pool(name="wtmp", bufs=1))
        wq_r = [wqk_pool.tile([128, H * P], f32r, tag=f"wq{c}", name=f"wq{c}") for c in range(DCH)]
        wk_r = [wqk_pool.tile([128, H * P], f32r, tag=f"wk{c}", name=f"wk{c}") for c in range(DCH)]
        ones_row = wtmp_pool.tile([1, S2], bf16, tag="ones", name="ones_row")
        nc.vector.memset(ones_row[:], 1.0)
        for h in range(H):
            nc.scalar.copy(khT[h][64:65, :], ones_row[:])

        def transpose_round(src_d, nrows, dstT):
            """src_d [nrows, D] fp32 DRAM -> dstT[c] [128, nrows] fp32r (c = d-chunk)."""
            with tc.tile_pool(name="tp_ps", bufs=2, space="PSUM") as tp_ps:
                ntile = nrows // 128
                for g in range(ntile // 4):
                    nats = []
                    for j in range(4):
                        si = 4 * g + j
                        nat = nat_pool.tile([128, D], f32, tag="nat")
                        nc.sync.dma_start(nat[:], src_d[si * 128:(si + 1) * 128, :])
                        nats.append(nat)
                    for c in range(DCH):
                        ps = tp_ps.tile([128, 512], f32)
                        for j in range(4):
                            nc.tensor.transpose(
                                ps[:, j * 128:(j + 1) * 128],
                                nats[j][:, c * 128:(c + 1) * 128],
                                ident[:],
                            )
                        eng = nc.vector.tensor_copy if c % 2 == 0 else nc.scalar.copy
                        eng(dstT[c][:, g * 512:(g + 1) * 512], ps[:])

        def load_w(name_d, dst):
            for c in range(DCH):
                wt = wtmp_pool.tile([128, H * P], f32, tag=f"wt{c}", name=f"wt{c}")
                nc.sync.dma_start(
                    wt[:].rearrange("p (h x) -> p h x", h=H),
                    name_d[:, c * 128:(c + 1) * 128, :].transpose([1, 0, 2]),
                )
                nc.gpsimd.tensor_copy(dst[c][:], wt[:])

        # ---- q path ----
        with tc.tile_pool(name="qT", bufs=1) as qT_pool, \
             tc.tile_pool(name="proj_ps", bufs=3, space="PSUM") as proj_ps:
            qT = [qT_pool.tile([128, SC], f32r, tag=f"qT{c}", name=f"qT{c}") for c in range(DCH)]
            transpose_round(q_d, SC, qT)
            load_w(wq_d, wq_r)
            for hp in range(H // 2):
                for sb in range(SBL):
                    ps = proj_ps.tile([128, 512], f32)
                    for c in range(DCH):
                        nc.tensor.matmul(
                            ps[:],
                            wq_r[c][:, hp * 128:(hp + 1) * 128],
                            qT[c][:, sb * 512:(sb + 1) * 512],
                            start=(c == 0), stop=(c == DCH - 1),
                        )
                    eng = nc.scalar.copy if sb == 0 else nc.vector.tensor_copy
                    eng(qhT[2 * hp][0:64, sb * 512:(sb + 1) * 512], ps[0:64, :])
                    eng(qhT[2 * hp + 1][0:64, sb * 512:(sb + 1) * 512], ps[64:128, :])

        # ---- k path ----
        with tc.tile_pool(name="kT", bufs=1) as kT_pool, \
             tc.tile_pool(name="proj_ps2", bufs=3, space="PSUM") as proj_ps:
            kT = [kT_pool.tile([128, S2], f32r, tag=f"kT{c}", name=f"kT{c}") for c in range(DCH)]
            transpose_round(k_d, S2, kT)
            load_w(wk_d, wk_r)
            load_w(wv_d, wv_r)
            for c in range(DCH):
                wt = wtmp_pool.tile([128, D], f32, tag=f"wt{c}", name=f"wto{c}")
                nc.sync.dma_start(wt[:], wo_d[c * 128:(c + 1) * 128, :])
                nc.gpsimd.tensor_copy(wo_bf[c][:], wt[:])
            for hp in range(H // 2):
                for tb in range(TBLK):
                    ps = proj_ps.tile([128, 512], f32)
                    for c in range(DCH):
                        nc.tensor.matmul(
                            ps[:],
                            wk_r[c][:, hp * 128:(hp + 1) * 128],
                            kT[c][:, tb * 512:(tb + 1) * 512],
                            start=(c == 0), stop=(c == DCH - 1),
                        )
                    eng = nc.scalar.copy if tb % 2 == 0 else nc.vector.tensor_copy
                    eng(khT[2 * hp][0:64, tb * 512:(tb + 1) * 512], ps[0:64, :])
                    eng(khT[2 * hp + 1][0:64, tb * 512:(tb + 1) * 512], ps[64:128, :])
        wqk_es.close()

        # ---- hybrid max pass machinery ----
        # t-half-0 ([0,1024)): raw S [s,t] chunks, DVE rowmax (negated).
        # t-half-1 ([1024,2048)): raw S^T [t,s] tiles, Pool partition-max
        # (axis=C) into stack rows; one stacked Pool reduce -> -max strip.
        # One slot per ti: raw-S [s,t] psum [128,1024] half, DVE negated rowmax;
        # after the odd half, min-combine (fused f32r cast) and DMA the
        # [128,1]->[1,128] strip into qhT's aug row.
        def mp_begin(h):
            return {}

        def mp_step(h, mp, ti, pool, ptag):
            if ti >= 2 * QT:
                return
            qi, half = ti // 2, ti % 2
            ps = pool.tile([128, 1024], f32, tag=ptag, name=f"mx{h}_{qi}_{half}")
            for tb in range(2):
                nc.tensor.matmul(
                    ps[:, tb * 512:(tb + 1) * 512],
                    qhT[h][0:64, qi * 128:(qi + 1) * 128],
                    khT[h][0:64, (2 * half + tb) * 512:(2 * half + tb + 1) * 512],
                    start=True, stop=True,
                )
            dst = small_pool.tile([128, 1], f32, tag=f"ng{half}", name=f"ng{half}_{h}_{qi}")
            nc.vector.tensor_reduce(
                dst[:], ps[:], axis=mybir.AxisListType.X,
                op=mybir.AluOpType.max, negate=True,
            )
            mp[(qi, half)] = dst
            if half == 1:
                negm_r = small_pool.tile([128, 1], f32r, tag="negmr", name=f"negmr{h}_{qi}")
                nc.gpsimd.tensor_scalar_min(negm_r[:], mp[(qi, 0)][:], dst[:])
                nc.sync.dma_start(
                    qhT[h][64:65, qi * 128:(qi + 1) * 128], negm_r[:],
                )

        # ---- v path (head 0 max pass interleaved) ----
        mp0 = mp_begin(0)
        with tc.tile_pool(name="vT", bufs=1) as vT_pool, \
             tc.tile_pool(name="mx0_ps", bufs=1, space="PSUM") as mx0_ps, \
             tc.tile_pool(name="proj_ps3", bufs=3, space="PSUM") as proj_ps:
            vT = [vT_pool.tile([128, S2], f32r, tag=f"vT{c}", name=f"vT{c}") for c in range(DCH)]
            transpose_round(v_d, S2, vT)
            for ti in range(TT):
                ps = proj_ps.tile([128, 512], f32)
                for c in range(DCH):
                    nc.tensor.matmul(
                        ps[:],
                        vT[c][:, ti * 128:(ti + 1) * 128],
                        wv_r[c][:],
                        start=(c == 0), stop=(c == DCH - 1),
                    )
                eng = nc.scalar.copy
                eng(
                    vh[ti].rearrange("p (h x) -> p h x", h=H)[:, :, 0:P],
                    ps[:].rearrange("p (h x) -> p h x", h=H),
                )
                mp_step(0, mp0, ti, mx0_ps, "mx0")
        ppool_es.close()

        # ---- attention per head ----
        fin_pool = ctx.enter_context(tc.tile_pool(name="fin", bufs=1))
        out_norm = [fin_pool.tile([128, H * P], bf16, tag=f"onorm{sc}", name=f"onorm{sc}") for sc in range(QT)]
        outT = [fin_pool.tile([128, SC], bf16, tag=f"outT{c}", name=f"outT{c}") for c in range(DCH)]

        with tc.tile_pool(name="stmx_ps", bufs=3, space="PSUM") as stmx_ps, \
             tc.tile_pool(name="oT_ps", bufs=2, space="PSUM") as oT_ps, \
             tc.tile_pool(name="pt", bufs=3) as pt_pool, \
             tc.tile_pool(name="oT_sb", bufs=2) as oT_sb_pool:
            for h in range(H):
                # --- S^T + exp + PV, next head's max pass interleaved ---
                if h + 1 < H:
                    mp_next = mp_begin(h + 1)
                oTs = [oT_ps.tile([65, 512], f32, tag="oT", name=f"oT{h}_{_sb}") for _sb in range(SBL)]
                for ti in range(TT):
                    st = stmx_ps.tile([128, 1024], f32, tag="stmx", name=f"st{h}_{ti}")
                    for sb in range(SBL):
                        nc.tensor.matmul(
                            st[:, sb * 512:(sb + 1) * 512],
                            khT[h][0:65, ti * 128:(ti + 1) * 128],
                            qhT[h][0:65, sb * 512:(sb + 1) * 512],
                            start=True, stop=True,
                        )
                    ptile = pt_pool.tile([128, 1024], bf16, tag="pt", name=f"pt{h}_{ti}")
                    nc.scalar.activation(ptile[:], st[:], mybir.ActivationFunctionType.Exp,
                                         scale=0.125)
                    for sb in range(SBL):
                        nc.tensor.matmul(
                            oTs[sb][0:65, :],
                            vh[ti][:, h * PA:(h + 1) * PA],
                            ptile[:, sb * 512:(sb + 1) * 512],
                            start=(ti == 0), stop=(ti == TT - 1),
                        )
                    if h + 1 < H:
                        mp_step(h + 1, mp_next, ti, stmx_ps, "stmx")

                # --- stage oT, reciprocal of rowsum row, transpose, normalize ---
                # pipelined per 512-col half so PE transposes of half 0 overlap
                # the copy+reciprocal of half 1; reciprocal reads the psum
                # rowsum row directly so it runs concurrently with the copy
                oT_h = oT_sb_pool.tile([65, SC], f32, tag="oT_h", name=f"oTh{h}")
                for sb in range(SBL):
                    sl = slice(sb * 512, (sb + 1) * 512)
                    nc.scalar.copy(oT_h[0:64, sl], oTs[sb][0:64, :])
                    nc.vector.reciprocal(oT_h[64:65, sl], oTs[sb][64:65, :])
                    for sc in range(sb * (QT // 2), (sb + 1) * (QT // 2)):
                        tps = oT_ps.tile([128, 128], f32, tag="oT", name=f"tps{h}_{sc}")
                        nc.tensor.transpose(
                            tps[0:128, 0:65],
                            oT_h[:, sc * 128:(sc + 1) * 128],
                            ident[0:65, 0:65],
                        )
                        nc.vector.tensor_scalar_mul(
                            out_norm[sc][:, h * P:(h + 1) * P], tps[:, 0:64], tps[:, 64:65],
                        )
                        if h % 2 == 1:
                            c = h // 2
                            tps2 = oT_ps.tile([128, 128], bf16, tag="oT", name=f"tb{c}_{sc}")
                            nc.tensor.transpose(
                                tps2[:], out_norm[sc][:, c * 128:(c + 1) * 128], ident_bf[:],
                            )
                            eng = nc.vector.tensor_copy if sc % 2 == 0 else nc.scalar.copy
                            eng(outT[c][:, sc * 128:(sc + 1) * 128], tps2[:])

        # ---- final projection ----
        with tc.tile_pool(name="fin_ps", bufs=2, space="PSUM") as fin_ps, \
             tc.tile_pool(name="fin_sb", bufs=2) as fin_sb_pool:
            for sc in range(QT):
                ps = fin_ps.tile([128, 512], f32, tag="fps", name=f"fps{sc}")
                for c in range(DCH):
                    nc.tensor.matmul(
                        ps[:],
                        outT[c][:, sc * 128:(sc + 1) * 128],
                        wo_bf[c][:],
                        start=(c == 0), stop=(c == DCH - 1),
                    )
                fin = fin_sb_pool.tile([128, 512], f32, tag="fin", name=f"fin{sc}")
                eng = nc.vector.tensor_copy if sc % 2 == 0 else nc.scalar.copy
                eng(fin[:], ps[:])
                nc.sync.dma_start(out_d[sc * 128:(sc + 1) * 128, :], fin[:])

    nc.compile()
    return nc


def kernel(q, k, v, Wq, Wk, Wv, Wo):
    nc = _build()
    from concourse.bass_utils import run_bass_kernel_spmd

    q = np.asarray(q, np.float32)
    k = np.asarray(k, np.float32)
    v = np.asarray(v, np.float32)
    in_maps = []
    for c in range(NCORES):
        b, half = c // 2, c % 2
        in_maps.append({
            "q": np.ascontiguousarray(q[b, half * SC:(half + 1) * SC, :]),
            "k": np.ascontiguousarray(k[b]),
            "v": np.ascontiguousarray(v[b]),
            "Wq": np.ascontiguousarray(Wq, dtype=np.float32),
            "Wk": np.ascontiguousarray(Wk, dtype=np.float32),
            "Wv": np.ascontiguousarray(Wv, dtype=np.float32),
            "Wo": np.ascontiguousarray(Wo, dtype=np.float32),
        })
    res = run_bass_kernel_spmd(nc, in_maps, core_ids=list(range(NCORES)))
    globals()["LAST_RES"] = res
    out = np.empty((B, S1, D), np.float32)
    for c, r in enumerate(res.results):
        b, half = c // 2, c % 2
        out[b, half * SC:(half + 1) * SC] = r["out"]
    return out


if __name__ == "__main__":
    rng = np.random.default_rng(0)
    qq = rng.standard_normal((B, S1, D), dtype=np.float32)
    kk = rng.standard_normal((B, S2, D), dtype=np.float32)
    vv = rng.standard_normal((B, S2, D), dtype=np.float32)
    wq = rng.standard_normal((H, D, P), dtype=np.float32)
    wk = rng.standard_normal((H, D, P), dtype=np.float32)
    wv = rng.standard_normal((H, D, P), dtype=np.float32)
    wo = rng.standard_normal((H * P, D), dtype=np.float32)
    o = kernel(qq, kk, vv, wq, wk, wv, wo)
    print("out", o.shape, o.dtype, np.abs(o).mean())


# revision 28
# speedup vs baseline: 1.3295x; 1.0157x over previous
"""CrossAttention TRN2 kernel: 8-core SPMD, shard = (batch, S1-half).

Per core: q rows [1024, 512] of one batch; full k,v [2048,512] of that batch;
all weights. Flash-style attention with S^T recompute (no P transpose):
  1. PE-transpose q,k,v -> qT,kT,vT (d on partitions), round to fp32r.
  2. Projections (fp32r): qhT_aug[65,1024]/khT_aug[65,2048] per head,
     vh_aug[t, 8*65] bf16 (per-head 64 cols + ones col for rowsum).
  3. Per head: raw-S max pass ([s,t] psum in 4 512-col chunks, rowmax
     reduces split Pool/DVE, negated, min-combined) -> aug row of qhT
     (-max, via HWDGE sbuf-to-sbuf DMA); S^T pass with K=65 (ones row in
     khT adds -max[s]); ACT exp(scale=1/8) -> P^T bf16; PV with augmented
     vh lhsT [128,65] accumulating oT[65, s] psum over 16 t-tiles (row 64
     = rowsum, no extra matmul).
  4. Per head: reciprocal of rowsum row in oT_h; transpose oT -> [s, 65]
     (col 64 = 1/rowsum); tensor_scalar_mul normalize into out_norm
     [s, 512]; transpose back -> outT [hp, s]; final proj vs Wo (bf16).
Engine budget: PE ~233us floor; exp on ACT only; copies/reduces split
Pool/DVE to keep both under PE.
"""
import sys
import functools

sys.path.insert(0, "/opt/trn_rl_repo")
import numpy as np
from contextlib import ExitStack

B, S1, S2, D, H, P = 4, 2048, 2048, 512, 8, 64
SC = S1 // 2          # 1024 q rows per core
NCORES = 8
DCH = D // 128        # 4 d-chunks
QT = SC // 128        # 8 q s-tiles
TT = S2 // 128        # 16 t-tiles
TBLK = S2 // 512      # 4 t blocks of 512
SBL = SC // 512       # 2 s blocks of 512
PA = P + 1            # 65: per-head vh block width (64 vals + ones col)


@functools.lru_cache(maxsize=1)
def _build():
    from concourse import bacc, tile, mybir, masks

    f32 = mybir.dt.float32
    f32r = mybir.dt.float32r
    bf16 = mybir.dt.bfloat16

    nc = bacc.Bacc("TRN2", target_bir_lowering=False, debug=False)

    q_d = nc.dram_tensor("q", [SC, D], f32, kind="ExternalInput").ap()
    k_d = nc.dram_tensor("k", [S2, D], f32, kind="ExternalInput").ap()
    v_d = nc.dram_tensor("v", [S2, D], f32, kind="ExternalInput").ap()
    wq_d = nc.dram_tensor("Wq", [H, D, P], f32, kind="ExternalInput").ap()
    wk_d = nc.dram_tensor("Wk", [H, D, P], f32, kind="ExternalInput").ap()
    wv_d = nc.dram_tensor("Wv", [H, D, P], f32, kind="ExternalInput").ap()
    wo_d = nc.dram_tensor("Wo", [H * P, D], f32, kind="ExternalInput").ap()
    out_d = nc.dram_tensor("out", [SC, D], f32, kind="ExternalOutput").ap()

    with tile.TileContext(nc) as tc, ExitStack() as ctx:
        const_pool = ctx.enter_context(tc.tile_pool(name="const", bufs=1))
        ident = const_pool.tile([128, 128], f32)
        masks.make_identity(nc, ident[:])
        ident_bf = const_pool.tile([128, 128], bf16)
        masks.make_identity(nc, ident_bf[:])

        # ---- weights: load fp32, round to fp32r / cast bf16 on gpsimd ----
        # wv_r/wo_bf persist (used in v path / final proj); wq_r/wk_r live in
        # a scoped pool freed before the v path to fit vT in SBUF.
        wpool = ctx.enter_context(tc.tile_pool(name="wr", bufs=1))
        wv_r = [wpool.tile([128, H * P], f32r, tag=f"wv{c}", name=f"wv{c}") for c in range(DCH)]
        wo_bf = [wpool.tile([128, D], bf16, tag=f"wo{c}", name=f"wo{c}") for c in range(DCH)]
        act_pool = ctx.enter_context(tc.tile_pool(name="acts", bufs=1))
        qhT = [act_pool.tile([65, SC], f32r, tag=f"qhT{h}", name=f"qhT{h}") for h in range(H)]
        khT = [act_pool.tile([65, S2], f32r, tag=f"khT{h}", name=f"khT{h}") for h in range(H)]
        vh = [act_pool.tile([128, H * PA], bf16, tag=f"vh{t}", name=f"vh{t}") for t in range(TT)]
        # ones columns of augmented vh (col 64 of each per-head 65-block)
        for t in range(TT):
            nc.gpsimd.memset(
                vh[t].rearrange("p (h x) -> p h x", h=H)[:, :, P:PA], 1.0,
            )
        # max-pass pools (persist into the attention phase) must be created
        # before the prologue-scoped pools for LIFO pool release
        small_pool = ctx.enter_context(tc.tile_pool(name="small", bufs=8))
        # prologue-scoped pools, closed explicitly to free SBUF for later phases
        ppool_es = ExitStack()
        nat_pool = ppool_es.enter_context(tc.tile_pool(name="nat", bufs=10))
        wqk_es = ExitStack()
        wqk_pool = wqk_es.enter_context(tc.tile_pool(name="wqk", bufs=1))
        wtmp_pool = wqk_es.enter_context(tc.tile_pool(name="wtmp", bufs=1))
        wq_r = [wqk_pool.tile([128, H * P], f32r, tag=f"wq{c}", name=f"wq{c}") for c in range(DCH)]
        wk_r = [wqk_pool.tile([128, H * P], f32r, tag=f"wk{c}", name=f"wk{c}") for c in range(DCH)]
        ones_row = wtmp_pool.tile([1, S2], bf16, tag="ones", name="ones_row")
        nc.vector.memset(ones_row[:], 1.0)
        for h in range(H):
            nc.scalar.copy(khT[h][64:65, :], ones_row[:])

        def transpose_round(src_d, nrows, dstT):
            """src_d [nrows, D] fp32 DRAM -> dstT[c] [128, nrows] fp32r (c = d-chunk)."""
            with tc.tile_pool(name="tp_ps", bufs=2, space="PSUM") as tp_ps:
                ntile = nrows // 128
                for g in range(ntile // 4):
                    nats = []
                    for j in range(4):
                        si = 4 * g + j
                        nat = nat_pool.tile([128, D], f32, tag="nat")
                        nc.sync.dma_start(nat[:], src_d[si * 128:(si + 1) * 128, :])
                        nats.append(nat)
                    for c in range(DCH):
                        ps = tp_ps.tile([128, 512], f32)
                        for j in range(4):
                            nc.tensor.transpose(
                                ps[:, j * 128:(j + 1) * 128],
                                nats[j][:, c * 128:(c + 1) * 128],
                                ident[:],
                            )
                        eng = nc.vector.tensor_copy if c % 2 == 0 else nc.scalar.copy
                        eng(dstT[c][:, g * 512:(g + 1) * 512], ps[:])

        def load_w(name_d, dst):
            for c in range(DCH):
                wt = wtmp_pool.tile([128, H * P], f32, tag=f"wt{c}", name=f"wt{c}")
                nc.sync.dma_start(
                    wt[:].rearrange("p (h x) -> p h x", h=H),
                    name_d[:, c * 128:(c + 1) * 128, :].transpose([1, 0, 2]),
                )
                nc.gpsimd.tensor_copy(dst[c][:], wt[:])

        # ---- q path ----
        with tc.tile_pool(name="qT", bufs=1) as qT_pool, \
             tc.tile_pool(name="proj_ps", bufs=3, space="PSUM") as proj_ps:
            qT = [qT_pool.tile([128, SC], f32r, tag=f"qT{c}", name=f"qT{c}") for c in range(DCH)]
            transpose_round(q_d, SC, qT)
            load_w(wq_d, wq_r)
            for hp in range(H // 2):
                for sb in range(SBL):
                    ps = proj_ps.tile([128, 512], f32)
                    for c in range(DCH):
                        nc.tensor.matmul(
                            ps[:],
                            wq_r[c][:, hp * 128:(hp + 1) * 128],
                            qT[c][:, sb * 512:(sb + 1) * 512],
                            start=(c == 0), stop=(c == DCH - 1),
                        )
                    eng = nc.scalar.copy if sb == 0 else nc.vector.tensor_copy
                    eng(qhT[2 * hp][0:64, sb * 512:(sb + 1) * 512], ps[0:64, :])
                    eng(qhT[2 * hp + 1][0:64, sb * 512:(sb + 1) * 512], ps[64:128, :])

        # ---- k path ----
        with tc.tile_pool(name="kT", bufs=1) as kT_pool, \
             tc.tile_pool(name="proj_ps2", bufs=3, space="PSUM") as proj_ps:
            kT = [kT_pool.tile([128, S2], f32r, tag=f"kT{c}", name=f"kT{c}") for c in range(DCH)]
            transpose_round(k_d, S2, kT)
            load_w(wk_d, wk_r)
            load_w(wv_d, wv_r)
            for c in range(DCH):
                wt = wtmp_pool.tile([128, D], f32, tag=f"wt{c}", name=f"wto{c}")
                nc.sync.dma_start(wt[:], wo_d[c * 128:(c + 1) * 128, :])
                nc.gpsimd.tensor_copy(wo_bf[c][:], wt[:])
            for hp in range(H // 2):
                for tb in range(TBLK):
                    ps = proj_ps.tile([128, 512], f32)
                    for c in range(DCH):
                        nc.tensor.matmul(
                            ps[:],
                            wk_r[c][:, hp * 128:(hp + 1) * 128],
                            kT[c][:, tb * 512:(tb + 1) * 512],
                            start=(c == 0), stop=(c == DCH - 1),
                        )
                    eng = nc.scalar.copy if tb % 2 == 0 else nc.vector.tensor_copy
                    eng(khT[2 * hp][0:64, tb * 512:(tb + 1) * 512], ps[0:64, :])
                    eng(khT[2 * hp + 1][0:64, tb * 512:(tb + 1) * 512], ps[64:128, :])
        wqk_es.close()

        # ---- hybrid max pass machinery ----
        # t-half-0 ([0,1024)): raw S [s,t] chunks, DVE rowmax (negated).
        # t-half-1 ([1024,2048)): raw S^T [t,s] tiles, Pool partition-max
        # (axis=C) into stack rows; one stacked Pool reduce -> -max strip.
        # One slot per ti: raw-S [s,t] psum [128,1024] half, DVE negated rowmax;
        # after the odd half, min-combine (fused f32r cast) and DMA the
        # [128,1]->[1,128] strip into qhT's aug row.
        def mp_begin(h):
            return {}

        def mp_step(h, mp, ti, pool, ptag):
            if ti >= 2 * QT:
                return
            qi, half = ti // 2, ti % 2
            ps = pool.tile([128, 1024], f32, tag=ptag, name=f"mx{h}_{qi}_{half}")
            for tb in range(2):
                nc.tensor.matmul(
                    ps[:, tb * 512:(tb + 1) * 512],
                    qhT[h][0:64, qi * 128:(qi + 1) * 128],
                    khT[h][0:64, (2 * half + tb) * 512:(2 * half + tb + 1) * 512],
                    start=True, stop=True,
                )
            dst = small_pool.tile([128, 1], f32, tag=f"ng{half}", name=f"ng{half}_{h}_{qi}")
            nc.vector.tensor_reduce(
                dst[:], ps[:], axis=mybir.AxisListType.X,
                op=mybir.AluOpType.max, negate=True,
            )
            mp[(qi, half)] = dst
            if half == 1:
                negm_r = small_pool.tile([128, 1], f32r, tag="negmr", name=f"negmr{h}_{qi}")
                nc.gpsimd.tensor_scalar_min(negm_r[:], mp[(qi, 0)][:], dst[:])
                nc.sync.dma_start(
                    qhT[h][64:65, qi * 128:(qi + 1) * 128], negm_r[:],
                )

        # ---- v path (head 0 max pass interleaved) ----
        mp0 = mp_begin(0)
        with tc.tile_pool(name="vT", bufs=1) as vT_pool, \
             tc.tile_pool(name="mx0_ps", bufs=1, space="PSUM") as mx0_ps, \
             tc.tile_pool(name="proj_ps3", bufs=3, space="PSUM") as proj_ps:
            vT = [vT_pool.tile([128, S2], f32r, tag=f"vT{c}", name=f"vT{c}") for c in range(DCH)]
            transpose_round(v_d, S2, vT)
            for ti in range(TT):
                ps = proj_ps.tile([128, 512], f32)
                for c in range(DCH):
                    nc.tensor.matmul(
                        ps[:],
                        vT[c][:, ti * 128:(ti + 1) * 128],
                        wv_r[c][:],
                        start=(c == 0), stop=(c == DCH - 1),
                    )
                eng = nc.scalar.copy
                eng(
                    vh[ti].rearrange("p (h x) -> p h x", h=H)[:, :, 0:P],
                    ps[:].rearrange("p (h x) -> p h x", h=H),
                )
                mp_step(0, mp0, ti, mx0_ps, "mx0")
        ppool_es.close()

        # ---- attention per head ----
        fin_pool = ctx.enter_context(tc.tile_pool(name="fin", bufs=1))
        out_norm = [fin_pool.tile([128, H * P], bf16, tag=f"onorm{sc}", name=f"onorm{sc}") for sc in range(QT)]
        outT = [fin_pool.tile([128, SC], bf16, tag=f"outT{c}", name=f"outT{c}") for c in range(DCH)]

        with tc.tile_pool(name="stmx_ps", bufs=3, space="PSUM") as stmx_ps, \
             tc.tile_pool(name="oT_ps", bufs=2, space="PSUM") as oT_ps, \
             tc.tile_pool(name="fin_sb", bufs=2) as fin_sb_pool, \
             tc.tile_pool(name="pt", bufs=3) as pt_pool, \
             tc.tile_pool(name="oT_sb", bufs=2) as oT_sb_pool:
            for h in range(H):
                # --- S^T + exp + PV, next head's max pass interleaved ---
                if h + 1 < H:
                    mp_next = mp_begin(h + 1)
                oTs = [oT_ps.tile([65, 512], f32, tag="oT", name=f"oT{h}_{_sb}") for _sb in range(SBL)]
                for ti in range(TT):
                    st = stmx_ps.tile([128, 1024], f32, tag="stmx", name=f"st{h}_{ti}")
                    for sb in range(SBL):
                        nc.tensor.matmul(
                            st[:, sb * 512:(sb + 1) * 512],
                            khT[h][0:65, ti * 128:(ti + 1) * 128],
                            qhT[h][0:65, sb * 512:(sb + 1) * 512],
                            start=True, stop=True,
                        )
                    ptile = pt_pool.tile([128, 1024], bf16, tag="pt", name=f"pt{h}_{ti}")
                    nc.scalar.activation(ptile[:], st[:], mybir.ActivationFunctionType.Exp,
                                         scale=0.125)
                    for sb in range(SBL):
                        nc.tensor.matmul(
                            oTs[sb][0:65, :],
                            vh[ti][:, h * PA:(h + 1) * PA],
                            ptile[:, sb * 512:(sb + 1) * 512],
                            start=(ti == 0), stop=(ti == TT - 1),
                        )
                    if h + 1 < H:
                        mp_step(h + 1, mp_next, ti, stmx_ps, "stmx")

                # --- stage oT, reciprocal of rowsum row, transpose, normalize ---
                # pipelined per 512-col half so PE transposes of half 0 overlap
                # the copy+reciprocal of half 1; reciprocal reads the psum
                # rowsum row directly so it runs concurrently with the copy
                oT_h = oT_sb_pool.tile([65, SC], f32, tag="oT_h", name=f"oTh{h}")
                for sb in range(SBL):
                    sl = slice(sb * 512, (sb + 1) * 512)
                    nc.scalar.copy(oT_h[0:64, sl], oTs[sb][0:64, :])
                    nc.vector.reciprocal(oT_h[64:65, sl], oTs[sb][64:65, :])
                    for sc in range(sb * (QT // 2), (sb + 1) * (QT // 2)):
                        tps = oT_ps.tile([128, 128], f32, tag="oT", name=f"tps{h}_{sc}")
                        nc.tensor.transpose(
                            tps[0:128, 0:65],
                            oT_h[:, sc * 128:(sc + 1) * 128],
                            ident[0:65, 0:65],
                        )
                        nc.vector.tensor_scalar_mul(
                            out_norm[sc][:, h * P:(h + 1) * P], tps[:, 0:64], tps[:, 64:65],
                        )
                        if h % 2 == 1:
                            c = h // 2
                            tps2 = oT_ps.tile([128, 128], bf16, tag="oT", name=f"tb{c}_{sc}")
                            nc.tensor.transpose(
                                tps2[:], out_norm[sc][:, c * 128:(c + 1) * 128], ident_bf[:],
                            )
                            eng = nc.vector.tensor_copy if sc % 2 == 0 else nc.scalar.copy
                            eng(outT[c][:, sc * 128:(sc + 1) * 128], tps2[:])
                            if h == H - 1:
                                # fused final projection: per-sc as soon as
                                # the last outT column block lands
                                fps = stmx_ps.tile([128, 1024], f32, tag="stmx", name=f"fps{sc}")
                                for cc in range(DCH):
                                    nc.tensor.matmul(
                                        fps[:, 0:512],
                                        outT[cc][:, sc * 128:(sc + 1) * 128],
                                        wo_bf[cc][:],
                                        start=(cc == 0), stop=(cc == DCH - 1),
                                    )
                                fin = fin_sb_pool.tile([128, 512], f32, tag="fin", name=f"fin{sc}")
                                feng = nc.vector.tensor_copy if sc % 2 == 0 else nc.scalar.copy
                                feng(fin[:], fps[:, 0:512])
                                nc.sync.dma_start(out_d[sc * 128:(sc + 1) * 128, :], fin[:])

    nc.compile()
    return nc


def kernel(q, k, v, Wq, Wk, Wv, Wo):
    nc = _build()
    from concourse.bass_utils import run_bass_kernel_spmd

    q = np.asarray(q, np.float32)
    k = np.asarray(k, np.float32)
    v = np.asarray(v, np.float32)
    in_maps = []
    for c in range(NCORES):
        b, half = c // 2, c % 2
        in_maps.append({
            "q": np.ascontiguousarray(q[b, half * SC:(half + 1) * SC, :]),
            "k": np.ascontiguousarray(k[b]),
            "v": np.ascontiguousarray(v[b]),
            "Wq": np.ascontiguousarray(Wq, dtype=np.float32),
            "Wk": np.ascontiguousarray(Wk, dtype=np.float32),
            "Wv": np.ascontiguousarray(Wv, dtype=np.float32),
            "Wo": np.ascontiguousarray(Wo, dtype=np.float32),
        })
    res = run_bass_kernel_spmd(nc, in_maps, core_ids=list(range(NCORES)))
    globals()["LAST_RES"] = res
    out = np.empty((B, S1, D), np.float32)
    for c, r in enumerate(res.results):
        b, half = c // 2, c % 2
        out[b, half * SC:(half + 1) * SC] = r["out"]
    return out


if __name__ == "__main__":
    rng = np.random.default_rng(0)
    qq = rng.standard_normal((B, S1, D), dtype=np.float32)
    kk = rng.standard_normal((B, S2, D), dtype=np.float32)
    vv = rng.standard_normal((B, S2, D), dtype=np.float32)
    wq = rng.standard_normal((H, D, P), dtype=np.float32)
    wk = rng.standard_normal((H, D, P), dtype=np.float32)
    wv = rng.standard_normal((H, D, P), dtype=np.float32)
    wo = rng.standard_normal((H * P, D), dtype=np.float32)
    o = kernel(qq, kk, vv, wq, wk, wv, wo)
    print("out", o.shape, o.dtype, np.abs(o).mean())
